# revision 1
# baseline (speedup 1.0000x reference)
"""Trainium2 Bass kernel for nn_CrossAttention (q-aware per-query V cross attention).

Reference computation (b=4, nq=64, n=1024, d=768, h=8, dh=96, R=64):
    q   = x @ Wq
    k   = context @ Wk
    h1  = LayerNorm(context @ Wv1)            # over the 4096 (= nq*R) axis
    vmid= h1.reshape(b, n, nq, R)
    v   = einsum('bnqr,qrd->bqnd', vmid, Wc)
    attn= softmax(q·k / sqrt(dh))             # per head
    out = einsum('bhij,bhijd->bhid', attn, v) @ Wout

Key algebraic restructuring used here: contract attn with vmid FIRST
(t[b,i,h,r] = sum_j attn[b,h,i,j] * vmid[b,j,i,r]), then apply the grouped
conv Wc and Wout on the tiny rank-space result. This avoids materializing
the 805MB v tensor and collapses ~52 GFLOP to ~6 GFLOP.

Sharding: the context axis n is split 8 ways (128 rows per batch per core).
Each core computes its local-j partial sums of (t, sumexp, mu-correction),
a ReduceScatter(add) over the query axis hands each core 8 queries' totals,
and the per-query tail (Wc grouped conv + Wout) is query-sharded.

LayerNorm folding: with e2 = exp(scores)*rstd (rstd folded into the exp bias
as ln(rstd)), t_z = sum_j e2*h1 - (sum_j e2*mu), sum_j e = sum_j e2*(1/rstd).
The 1/rstd and mu columns are appended to the h1 tile so one extra matmul
yields both normalizers. gamma/beta are applied post-collective on t
(sum_j attn = 1).
"""

import json

import numpy as np

import concourse.bass as bass
import concourse.mybir as mybir
import concourse.tile as tile
from concourse.bass_utils import run_bass_kernel_spmd

F32 = mybir.dt.float32
F32R = mybir.dt.float32r
AF = mybir.ActivationFunctionType

B = 4
NQ = 64
N = 1024
D = 768
H = 8
DH = 96
R = 64
NQR = NQ * R  # 4096
LN_EPS = 1e-5
N_CORES = 8
NLOC = N // N_CORES  # 128 context rows per batch per core
QLOC = NQ // N_CORES  # 8 queries per core
KC = D // 128  # 6 contraction chunks of 128
QK_SCALE = float(DH) ** -0.5


class WaitSplitBass(bass.Bass):
    """This walrus build rejects instructions carrying more than one sync
    wait; split extras into preceding same-engine NoOps at JSON time."""

    MAX_WAITS = 1

    def to_json_bytes(self) -> bytes:
        raw = super().to_json_bytes()
        m = json.loads(raw)
        changed = False
        for f in m.get("functions", []):
            for blk in f.get("blocks", []):
                out = []
                for inst in blk.get("instructions", []):
                    si = inst.get("sync_info")
                    waits = si.get("on_wait") if si else None
                    if waits and len(waits) > self.MAX_WAITS:
                        extra = waits[self.MAX_WAITS:]
                        si["on_wait"] = waits[: self.MAX_WAITS]
                        for k, w in enumerate(extra):
                            out.append({
                                "engine": inst["engine"],
                                "ins": [],
                                "name": f"{inst['name']}_ws{k}",
                                "opcode": "NoOp",
                                "outs": [],
                                "sync_info": {"on_update": [], "on_wait": [w]},
                            })
                        changed = True
                    out.append(inst)
                blk["instructions"] = out
        return json.dumps(m).encode() if changed else raw


def _emit(nc, debug=False):
    x = nc.declare_dram_parameter("x", [B * NQ, D], F32, isOutput=False)
    ctx = nc.declare_dram_parameter("ctx", [B, NLOC, D], F32, isOutput=False)
    wq = nc.declare_dram_parameter("wq", [D, D], F32, isOutput=False)
    wk = nc.declare_dram_parameter("wk", [D, D], F32, isOutput=False)
    wv1 = nc.declare_dram_parameter("wv1", [D, NQR], F32, isOutput=False)
    wc = nc.declare_dram_parameter("wc", [QLOC, R, D], F32, isOutput=False)
    wout = nc.declare_dram_parameter("wout", [D, D], F32, isOutput=False)
    by = nc.declare_dram_parameter("by", [QLOC, D], F32, isOutput=False)
    y = nc.declare_dram_parameter("y", [B, QLOC, D], F32, isOutput=True)
    dbg = None
    if debug:
        dbg = {
            "dbg_tall": nc.declare_dram_parameter(
                "dbg_tall", [128, 4, B, 66], F32, isOutput=True),
            "dbg_tred": nc.declare_dram_parameter(
                "dbg_tred", [16, 4, B, 66], F32, isOutput=True),
            "dbg_tn": nc.declare_dram_parameter(
                "dbg_tn", [2, 128, R], F32, isOutput=True),
            "dbg_tnraw": nc.declare_dram_parameter(
                "dbg_tnraw", [2, 128, R], F32, isOutput=True),
            "dbg_scn": nc.declare_dram_parameter(
                "dbg_scn", [2, 128, 2], F32, isOutput=True),
            "dbg_tfT": nc.declare_dram_parameter(
                "dbg_tfT", [R, 256], F32, isOutput=True),
            "dbg_u": nc.declare_dram_parameter(
                "dbg_u", [DH, H, 32], F32, isOutput=True),
        }

    with tile.TileContext(nc) as tc:
        _body(nc, tc, x, ctx, wq, wk, wv1, wc, wout, by, y, dbg)
    return nc


def _body(nc, tc, x, ctx, wq, wk, wv1, wc, wout, by, y, dbg=None):
    from contextlib import ExitStack

    with ExitStack() as st:
        # long-lived pools (whole kernel)
        const = st.enter_context(tc.tile_pool(name="const", bufs=1))
        core = st.enter_context(tc.tile_pool(name="core", bufs=1))
        small = st.enter_context(tc.tile_pool(name="small", bufs=4))
        ps_h = st.enter_context(tc.tile_pool(name="ps_h", bufs=2, space="PSUM"))
        ps_m = st.enter_context(tc.tile_pool(name="ps_m", bufs=2, space="PSUM"))
        ps_t = st.enter_context(tc.tile_pool(name="ps_t", bufs=2, space="PSUM"))
        dram = st.enter_context(tc.tile_pool(name="dram", bufs=1, space="DRAM"))

        ident = const.tile([128, 128], F32)
        from concourse.masks import make_identity
        make_identity(nc, ident[:])
        eps_t = const.tile([128, 1], F32)
        nc.vector.memset(eps_t[:], LN_EPS)

        # core-resident tensors
        wv1_sb = [core.tile([128, NQR], F32R, tag=f"wv1{k}", name=f"wv1{k}")
                  for k in range(KC)]
        ctxT = [core.tile([128, B * NLOC], F32R, tag=f"cT{k}", name=f"cT{k}")
                for k in range(KC)]
        q_sb = [core.tile([DH, B * NQ], F32, tag=f"q{h}", name=f"q{h}")
                for h in range(H)]
        k_sb = [core.tile([DH, B * NLOC], F32, tag=f"k{h}", name=f"k{h}")
                for h in range(H)]

        # ---- phase A: load x/ctx, transpose, q/k head projections ----
        with tc.tile_pool(name="phaseA", bufs=1) as pa:
            wq_sb = [pa.tile([128, D], F32R, tag=f"wq{k}", name=f"wq{k}")
                     for k in range(KC)]
            wk_sb = [pa.tile([128, D], F32R, tag=f"wk{k}", name=f"wk{k}")
                     for k in range(KC)]
            for k in range(KC):
                nc.sync.dma_start(out=wq_sb[k][:],
                                  in_=wq[k * 128:(k + 1) * 128, :].bitcast(F32R))
                nc.sync.dma_start(out=wk_sb[k][:],
                                  in_=wk[k * 128:(k + 1) * 128, :].bitcast(F32R))
            x_sb = [pa.tile([128, D], F32, tag=f"x_in{r_}", name=f"x_in{r_}")
                    for r_ in range(2)]
            for r_ in range(2):
                nc.sync.dma_start(out=x_sb[r_][:], in_=x[r_ * 128:(r_ + 1) * 128, :])
            ctx_sb = [pa.tile([128, D], F32, tag=f"ctx_in{bb}", name=f"ctx_in{bb}")
                      for bb in range(B)]
            for bb in range(B):
                nc.sync.dma_start(out=ctx_sb[bb][:], in_=ctx[bb])
            xT = [pa.tile([128, B * NQ], F32R, tag=f"xT{k}", name=f"xT{k}")
                  for k in range(KC)]
            # wv1 is large and first consumed ~20us in; emit after the
            # latency-critical phase-A loads so it doesn't head-of-line
            # block the DMA queues
            for k in range(KC):
                nc.sync.dma_start(out=wv1_sb[k][:],
                                  in_=wv1[k * 128:(k + 1) * 128, :].bitcast(F32R))

            tr_n = 0
            for k in range(KC):
                for r_ in range(2):
                    pt = ps_m.tile([128, 128], F32, tag="m", name="m_ps")
                    nc.tensor.transpose(pt[:], x_sb[r_][:, k * 128:(k + 1) * 128],
                                        ident[:])
                    eng = nc.vector.tensor_copy if tr_n % 2 else nc.scalar.copy
                    eng(out=xT[k][:, r_ * 128:(r_ + 1) * 128], in_=pt[:])
                    tr_n += 1
                for bb in range(B):
                    pt = ps_m.tile([128, 128], F32, tag="m", name="m_ps")
                    nc.tensor.transpose(pt[:], ctx_sb[bb][:, k * 128:(k + 1) * 128],
                                        ident[:])
                    eng = nc.vector.tensor_copy if tr_n % 2 else nc.scalar.copy
                    eng(out=ctxT[k][:, bb * 128:(bb + 1) * 128], in_=pt[:])
                    tr_n += 1

            for h in range(H):
                qp = ps_m.tile([DH, B * NQ], F32, tag="m", name="m_ps")
                for k in range(KC):
                    nc.tensor.matmul(qp[:], wq_sb[k][:, h * DH:(h + 1) * DH], xT[k][:],
                                     start=(k == 0), stop=(k == KC - 1))
                nc.scalar.copy(out=q_sb[h][:], in_=qp[:])
                kp = ps_m.tile([DH, B * NLOC], F32, tag="m", name="m_ps")
                for k in range(KC):
                    nc.tensor.matmul(kp[:], wk_sb[k][:, h * DH:(h + 1) * DH],
                                     ctxT[k][:], start=(k == 0), stop=(k == KC - 1))
                nc.scalar.copy(out=k_sb[h][:], in_=kp[:])

        # ---- phase B: h1 + attention partial sums ----
        # Combined staging tensor: rows = (il 16, h 8), free = (ig 4, b 4,
        # rc 66) where rc = 64 t-values + (s, c). ReduceScatter chunks rows:
        # core c owns il in {2c, 2c+1} -> query ids {16*ig + 2c + m}.
        t_all = dram.tile([128, 4, B, 66], F32)
        with tc.tile_pool(name="phaseB", bufs=1) as pb:
            # SBUF staging partitions = (i_l 4, v 32), v < 8 (= h) is live;
            # compute-engine APs must start at partition 0/32/64/96, so
            # queries sit on 32-row boundaries here and the compaction DMAs
            # below re-pack to (il, h) rows.
            t2_stage = pb.tile([128, 16, B, 66], F32, tag="t2", name="t2")
            def emit_h1(bb):
                h1_t = pb.tile([128, NQR + 2], F32R, tag=f"h1_{bb % 2}",
                               name=f"h1_{bb % 2}")
                stats = small.tile([128, 8, 6], F32, tag="stats", name="stats")
                for nn in range(8):
                    hp = ps_h.tile([128, 512], F32, tag="h_ps", name="h_ps")
                    for k in range(KC):
                        nc.tensor.matmul(
                            hp[:], ctxT[k][:, bb * 128:(bb + 1) * 128],
                            wv1_sb[k][:, nn * 512:(nn + 1) * 512],
                            start=(k == 0), stop=(k == KC - 1))
                    nc.vector.bn_stats(out=stats[:, nn, :], in_=hp[:])
                    nc.scalar.copy(out=h1_t[:, nn * 512:(nn + 1) * 512], in_=hp[:])
                mv = small.tile([128, 2], F32, tag="mv", name="mv")
                nc.vector.bn_aggr(out=mv[:], in_=stats[:])
                # cols 4096/4097: 1/rstd = sqrt(var+eps), mu
                nc.scalar.activation(out=h1_t[:, NQR:NQR + 1], in_=mv[:, 1:2],
                                     func=AF.Sqrt, bias=eps_t[:])
                nc.vector.tensor_copy(out=h1_t[:, NQR + 1:NQR + 2], in_=mv[:, 0:1])
                lnr = small.tile([128, 1], F32, tag="lnr", name="lnr")
                nc.scalar.activation(out=lnr[:], in_=mv[:, 1:2], func=AF.Ln,
                                     bias=eps_t[:])
                nc.vector.tensor_scalar_mul(lnr[:], lnr[:], -0.5)
                return h1_t, lnr

            def emit_scores(bb, lnr):
                # e2 col = i*32 + h (h < 8; cols h >= 8 are never-read junk)
                e2 = pb.tile([128, NQ * 32], F32R, tag="e2", name="e2")
                e2v = e2[:].rearrange("p (i v) -> p i v", v=32)
                for h in range(H):
                    sp = ps_m.tile([128, NQ], F32, tag="m", name="m_ps")
                    nc.tensor.matmul(sp[:], k_sb[h][:, bb * 128:(bb + 1) * 128],
                                     q_sb[h][:, bb * NQ:(bb + 1) * NQ],
                                     start=True, stop=True)
                    nc.scalar.activation(out=e2v[:, :, h], in_=sp[:], func=AF.Exp,
                                         scale=QK_SCALE, bias=lnr[:])
                return e2

            def emit_t5(bb, h1_t, e2):
                # t_raw chunks: 4 queries per matmul, psum partition=(i_l, v32)
                for ic in range(16):
                    tp = ps_t.tile([128, 256], F32, tag="t_ps", name="t_ps")
                    lhs = e2[:, ic * 128:(ic + 1) * 128]
                    nc.tensor.matmul(tp[:], lhs,
                                     h1_t[:, ic * 256:(ic + 1) * 256],
                                     start=True, stop=True)
                    scp = ps_m.tile([128, 2], F32, tag="m", name="m_ps")
                    nc.tensor.matmul(scp[:], lhs, h1_t[:, NQR:NQR + 2],
                                     start=True, stop=True)
                    nc.vector.tensor_copy(out=t2_stage[:, ic, bb, 64:66],
                                          in_=scp[:])
                    for il in range(4):
                        src_ap = tp[il * 32:il * 32 + 8,
                                    il * 64:(il + 1) * 64]
                        dst_ap = t2_stage[il * 32:il * 32 + 8, ic, bb, 0:64]
                        if (ic % 2) == 1:
                            nc.scalar.copy(out=dst_ap, in_=src_ap)
                        else:
                            nc.vector.tensor_copy(out=dst_ap, in_=src_ap)

            # software pipeline: PE fills the stats->exp gap of batch bb with
            # h1 matmuls of batch bb+1
            h1_cur, lnr_cur = emit_h1(0)
            e2_cur = emit_scores(0, lnr_cur)
            for bb in range(B):
                if bb + 1 < B:
                    h1_nxt, lnr_nxt = emit_h1(bb + 1)
                emit_t5(bb, h1_cur, e2_cur)
                if bb + 1 < B:
                    e2_cur = emit_scores(bb + 1, lnr_nxt)
                    h1_cur = h1_nxt

            # compact (i_l, v32) staging into (il, h) DRAM rows; plain
            # slices only (partition-split rearranges on DMA operands are
            # silently wrong on this stack)
            for ic in range(16):
                for il in range(4):
                    i = ic * 4 + il
                    row = (i % 16) * 8
                    ig = i // 16
                    nc.sync.dma_start(
                        out=t_all[row:row + 8, ig, :, :],
                        in_=t2_stage[il * 32:il * 32 + 8, ic, :, :])

        # ---- ReduceScatter over query axis ----
        t_red = dram.tile([16, 4, B, 66], F32)
        nc.gpsimd.collective_compute(
            "ReduceScatter", mybir.AluOpType.add,
            replica_groups=[list(range(N_CORES))],
            ins=[t_all.opt()], outs=[t_red.opt()])

        if dbg is not None:
            nc.sync.dma_start(out=dbg["dbg_tall"][:], in_=t_all[:])
            nc.sync.dma_start(out=dbg["dbg_tred"][:], in_=t_red[:])

        # ---- phase C: tail (normalize, gamma/beta, Wc, Wout) ----
        with tc.tile_pool(name="phaseC", bufs=1) as pc:
            wc_sb = [pc.tile([R, D], F32, tag=f"wc{i}", name=f"wc{i}")
                     for i in range(QLOC)]
            for i in range(QLOC):
                nc.sync.dma_start(out=wc_sb[i][:], in_=wc[i])
            wout_sb = [pc.tile([DH, D], F32R, tag=f"wo{h}", name=f"wo{h}")
                       for h in range(H)]
            for h in range(H):
                nc.sync.dma_start(out=wout_sb[h][:],
                                  in_=wout[h * DH:(h + 1) * DH, :].bitcast(F32R))

            tnc = [pc.tile([128, 66], F32, tag=f"tnc{t}", name=f"tnc{t}")
                   for t in range(2)]
            by_sb = pc.tile([32, D], F32, tag="by_sb", name="by_sb")
            for il in range(QLOC):
                by_ap = bass.AP(tensor=by[:].tensor,
                                offset=by[:].offset + il * D,
                                ap=[[0, B], [1, D]])
                nc.sync.dma_start(out=by_sb[il * 4:(il + 1) * 4, :], in_=by_ap)
            # t_red rows = (m 2, h 8), free (ig, b, rc=66); m = tt.
            # tnc partition p = h*16 + ig*4 + b ; i_loc = tt*4 + ig.
            for tt in range(2):
                for h in range(H):
                    nc.sync.dma_start(
                        out=tnc[tt][h * 16:(h + 1) * 16, :],
                        in_=t_red[tt * 8 + h, :, :, :])
            tn = [tnc[t][:, 0:64] for t in range(2)]
            for tt in range(2):
                if dbg is not None:
                    nc.sync.dma_start(out=dbg["dbg_tnraw"][tt], in_=tn[tt])
                    nc.sync.dma_start(out=dbg["dbg_scn"][tt],
                                      in_=tnc[tt][:, 64:66])
                rcp = small.tile([128, 1], F32, tag="rcp", name="rcp")
                nc.vector.reciprocal(out=rcp[:], in_=tnc[tt][:, 64:65])
                nc.vector.tensor_scalar(
                    out=tn[tt], in0=tn[tt],
                    scalar1=tnc[tt][:, 65:66], scalar2=rcp[:],
                    op0=mybir.AluOpType.subtract, op1=mybir.AluOpType.mult)
                if dbg is not None:
                    nc.sync.dma_start(out=dbg["dbg_tn"][tt], in_=tn[tt])

            # transpose -> t_fT [r 64, (i8, b4, h8) 256]
            t_fT = pc.tile([R, 256], F32, tag="t_fT", name="t_fT")
            for tt in range(2):
                pt = ps_m.tile([128, 128], F32, tag="m", name="m_ps")
                nc.tensor.transpose(pt[:R, :], tn[tt], ident[:])
                nc.vector.tensor_copy(out=t_fT[:, tt * 128:(tt + 1) * 128],
                                      in_=pt[:R, :])

            # u[c, h, (i,b)] = sum_r Wc[i, r, h*96+c] * t_f[(i,b,h), r]
            up = ps_m.tile([DH, H, 32], F32, tag="m", name="m_ps")
            t_fTv = t_fT[:].rearrange("r (m h g b) -> r m h g b", m=2, h=H, g=4)
            for il in range(QLOC):
                tt, ig = il // 4, il % 4
                for h in range(H):
                    nc.tensor.matmul(
                        up[:, h, il * 4:(il + 1) * 4],
                        wc_sb[il][:, h * DH:(h + 1) * DH],
                        t_fTv[:, tt, h, ig, :],
                        start=True, stop=True)
            u_sb = pc.tile([DH, H, 32], F32R, tag="u_sb", name="u_sb")
            nc.vector.tensor_copy(out=u_sb[:], in_=up[:])
            if dbg is not None:
                nc.sync.dma_start(out=dbg["dbg_tfT"][:], in_=t_fT[:])
                nc.sync.dma_start(out=dbg["dbg_u"][:], in_=u_sb[:].bitcast(F32))

            # y[(i,b), e] = sum_h u[:, h, :]^T @ Wout[h*96:(h+1)*96, :]
            yp = ps_h.tile([32, D], F32, tag="h_ps", name="y_ps")
            for half, w in ((0, 512), (1, 256)):
                for h in range(H):
                    nc.tensor.matmul(
                        yp[:, half * 512: half * 512 + w],
                        u_sb[:, h, :],
                        wout_sb[h][:, half * 512: half * 512 + w],
                        start=(h == 0), stop=(h == H - 1))
            y_sb = pc.tile([32, D], F32, tag="y_sb", name="y_sb")
            nc.vector.tensor_add(y_sb[:], yp[:], by_sb[:])
            nc.sync.dma_start(out=y[:].rearrange("b i e -> i b e"),
                              in_=y_sb[:])



_CACHE = {}


def _get_nc():
    if "nc" not in _CACHE:
        nc = WaitSplitBass("TRN2", target_bir_lowering=False, debug=False,
                           num_devices=N_CORES)
        _CACHE["nc"] = _emit(nc)
    return _CACHE["nc"]


def core_query_ids(c):
    """Queries owned by core c after ReduceScatter, indexed by i_loc = m*4+ig."""
    return [16 * ig + 2 * c + m for m in range(2) for ig in range(4)]


def make_in_maps(x, context, Wq, Wk, Wv1, ln_g, ln_b, Wc, Wout):
    x = np.ascontiguousarray(x, dtype=np.float32).reshape(B * NQ, D)
    g2 = np.asarray(ln_g, dtype=np.float32).reshape(NQ, R)
    b2 = np.asarray(ln_b, dtype=np.float32).reshape(NQ, R)
    Wc = np.asarray(Wc, dtype=np.float32)
    Wout = np.asarray(Wout, dtype=np.float32)
    # fold LN gamma into Wc, and beta (x sum(attn)=1) through Wc@Wout into a
    # per-query output bias
    Wcg = g2[:, :, None] * Wc
    bias_y = np.einsum("ir,ird->id", b2, Wc) @ Wout
    maps = []
    for c in range(N_CORES):
        maps.append({
            "x": x,
            "ctx": np.ascontiguousarray(
                context[:, c * NLOC:(c + 1) * NLOC, :], dtype=np.float32),
            "wq": np.ascontiguousarray(Wq, dtype=np.float32),
            "wk": np.ascontiguousarray(Wk, dtype=np.float32),
            "wv1": np.ascontiguousarray(Wv1, dtype=np.float32),
            "wc": np.ascontiguousarray(Wcg[core_query_ids(c)]),
            "wout": np.ascontiguousarray(Wout, dtype=np.float32),
            "by": np.ascontiguousarray(bias_y[core_query_ids(c)]),
        })
    return maps


def assemble(results):
    # per-core y [B, QLOC, D] -> [B, NQ, D], scattered by core_query_ids
    out = np.empty((B, NQ, D), dtype=np.float32)
    for c in range(N_CORES):
        out[:, core_query_ids(c), :] = results[c]["y"]
    return out


def kernel(x, context, Wq, Wk, Wv1, ln_g, ln_b, Wc, Wout):
    nc = _get_nc()
    maps = make_in_maps(x, context, Wq, Wk, Wv1, ln_g, ln_b, Wc, Wout)
    res = run_bass_kernel_spmd(nc, maps, list(range(N_CORES)))
    return assemble(res.results).astype(np.float32)



# revision 8
# speedup vs baseline: 9.1145x; 9.1145x over previous
"""Trainium2 Bass kernel for nn_CrossAttention (q-aware per-query V cross attention).

Reference computation (b=4, nq=64, n=1024, d=768, h=8, dh=96, R=64):
    q   = x @ Wq
    k   = context @ Wk
    h1  = LayerNorm(context @ Wv1)            # over the 4096 (= nq*R) axis
    vmid= h1.reshape(b, n, nq, R)
    v   = einsum('bnqr,qrd->bqnd', vmid, Wc)
    attn= softmax(q·k / sqrt(dh))             # per head
    out = einsum('bhij,bhijd->bhid', attn, v) @ Wout

Key algebraic restructuring: contract attn with vmid FIRST
(t[b,i,h,r] = sum_j attn[b,h,i,j] * vmid[b,j,i,r]), then apply the grouped
conv Wc and Wout on the tiny rank-space result. This avoids materializing
the 805MB v tensor and collapses ~52 GFLOP to ~6 GFLOP.

This version is engineered for the axon-tunnel regime where host->device
transfer (~90MB/s marginal, ~7ms fixed per array) dominates wall clock.
Per-call wire traffic is cut from ~189MB to ~11MB:
  * every replicated tensor is sharded 1/8 per core (96 = D/8 rows each of
    q^T, Wk, Wv1) and AllGathered on-device over NeuronLink. The 96-row
    shard boundaries double as the matmul contraction chunking, so the
    gathered regions are consumed in place with zero repacking;
  * one packed u8 wire tensor per core ([qT | Wk | Wv1 | ctxT]) to pay the
    per-array fixed cost once; one staging DMA + one AllGather covers the
    first three regions, ctxT stays core-local;
  * bf16 on the wire for ctx/Wk/Wv1 (q^T stays f32 as the precision
    anchor of the score path). fp8 was measured at 2.7e-2 absmax-rel for
    either ctx or Wv1 (host-only ablation matches) vs 2.5e-3 for all-bf16,
    so bf16 is the wire floor;
  * the tiny endpoints run on host: q = x@Wq (151 MFLOP) before dispatch,
    and the rank-space tail (gamma/beta fold, grouped conv Wc, Wout:
    ~330 MFLOP) after summing the 8 partial t tensors (135KB each).

Device work per core: k head projections, h1 = ctx_loc @ Wv1 (25.8 GFLOP
fleet-wide), LN stats folded into the exp bias (e2 = exp(s)*rstd via
ln(rstd) bias; 1/rstd and mu appended as h1 columns so one matmul yields
both normalizers), and t = e2^T @ h1 partial sums over the local 128
context rows. Host reduces the 8 partials (the old on-device
ReduceScatter) and finishes the tail in numpy.
"""

import json
import os

import numpy as np
import ml_dtypes

import jax

# Fresh shard_map closures inside run_bass_kernel_spmd defeat jax's
# in-memory executable cache, so every call re-runs the BIR->NEFF pipeline
# (~0.35s). The persistent cache is keyed on the (stable) HLO hash and
# brings repeat calls down to a disk load.
jax.config.update("jax_compilation_cache_dir",
                  os.path.expanduser("~/.cache/jax_bass_cache"))
jax.config.update("jax_persistent_cache_min_entry_size_bytes", -1)
jax.config.update("jax_persistent_cache_min_compile_time_secs", 0.0)

import concourse.bass as bass
import concourse.mybir as mybir
import concourse.tile as tile
from concourse.bass_utils import run_bass_kernel_spmd

F32 = mybir.dt.float32
F32R = mybir.dt.float32r
BF16 = mybir.dt.bfloat16
FP8 = mybir.dt.float8e4
AF = mybir.ActivationFunctionType
NP_BF16 = ml_dtypes.bfloat16
NP_FP8 = ml_dtypes.float8_e4m3

B = 4
NQ = 64
N = 1024
D = 768
H = 8
DH = 96
R = 64
NQR = NQ * R  # 4096
LN_EPS = 1e-5
N_CORES = 8
NLOC = N // N_CORES  # 128 context rows per batch per core
QK_SCALE = float(DH) ** -0.5

# packed wire layout per core (bytes)
QT_BYTES = DH * B * NQ * 4          # 98304  f32 [96, 256]
WK_BYTES = DH * D * 2               # 147456 bf16 [96, 768]
WV1_BYTES = DH * NQR * 2            # 786432 bf16 [96, 4096]
GATH_BYTES = QT_BYTES + WK_BYTES + WV1_BYTES  # 1032192, AllGathered
CTX_BYTES = D * B * NLOC * 2        # 786432 bf16 [8, 96, 512], core-local
PACK_BYTES = GATH_BYTES + CTX_BYTES


class WaitSplitBass(bass.Bass):
    """This walrus build rejects instructions carrying more than one sync
    wait; split extras into preceding same-engine NoOps at JSON time."""

    MAX_WAITS = 1

    def to_json_bytes(self) -> bytes:
        raw = super().to_json_bytes()
        m = json.loads(raw)
        changed = False
        for f in m.get("functions", []):
            for blk in f.get("blocks", []):
                out = []
                for inst in blk.get("instructions", []):
                    si = inst.get("sync_info")
                    waits = si.get("on_wait") if si else None
                    if waits and len(waits) > self.MAX_WAITS:
                        extra = waits[self.MAX_WAITS:]
                        si["on_wait"] = waits[: self.MAX_WAITS]
                        for k, w in enumerate(extra):
                            out.append({
                                "engine": inst["engine"],
                                "ins": [],
                                "name": f"{inst['name']}_ws{k}",
                                "opcode": "NoOp",
                                "outs": [],
                                "sync_info": {"on_update": [], "on_wait": [w]},
                            })
                        changed = True
                    out.append(inst)
                blk["instructions"] = out
        return json.dumps(m).encode() if changed else raw


def _emit(nc):
    packed = nc.declare_dram_parameter("packed", [PACK_BYTES], mybir.dt.uint8,
                                       isOutput=False)
    # ReduceScattered t sums: rows (m 2, h 8) -> query i = 16*ig + 2*core + m
    tout = nc.declare_dram_parameter("tout", [16, 4, B, 66], F32,
                                     isOutput=True)
    with tile.TileContext(nc) as tc:
        _body(nc, tc, packed, tout)
    return nc


def _view(t, byte_off, dtype, rows, cols):
    """2-D [rows, cols] view of a byte range of the flat u8 tensor t."""
    esz = np.dtype(mybir.dt.np(dtype)).itemsize
    ap = t[byte_off:byte_off + rows * cols * esz].bitcast(dtype)
    return ap.rearrange("(a b) -> a b", a=rows)


def _body(nc, tc, packed, tout):
    from contextlib import ExitStack

    with ExitStack() as st:
        const = st.enter_context(tc.tile_pool(name="const", bufs=1))
        core = st.enter_context(tc.tile_pool(name="core", bufs=1))
        small = st.enter_context(tc.tile_pool(name="small", bufs=4))
        ps_h = st.enter_context(tc.tile_pool(name="ps_h", bufs=2, space="PSUM"))
        ps_m = st.enter_context(tc.tile_pool(name="ps_m", bufs=2, space="PSUM"))
        ps_t = st.enter_context(tc.tile_pool(name="ps_t", bufs=2, space="PSUM"))
        dram = st.enter_context(tc.tile_pool(name="dram", bufs=1, space="DRAM"))

        eps_t = const.tile([128, 1], F32)
        nc.vector.memset(eps_t[:], LN_EPS)

        # ---- one staged copy + one AllGather of the [qT|Wk|Wv1] regions ----
        # (collectives cannot read IO tensors, hence the staging DMA)
        s_all = dram.tile([GATH_BYTES], mybir.dt.uint8)
        g_all = dram.tile([N_CORES * GATH_BYTES], mybir.dt.uint8,
                          addr_space="Shared")
        nc.sync.dma_start(out=s_all[:], in_=packed[0:GATH_BYTES])
        nc.gpsimd.collective_compute(
            "AllGather", mybir.AluOpType.bypass,
            replica_groups=[list(range(N_CORES))],
            ins=[s_all[:].opt()], outs=[g_all[:].opt()])

        # core-resident tensors
        wv1_sb = [core.tile([DH, NQR], BF16, tag=f"wv1{c}", name=f"wv1{c}")
                  for c in range(N_CORES)]
        ctxT = [core.tile([DH, B * NLOC], BF16, tag=f"cT{c}", name=f"cT{c}")
                for c in range(N_CORES)]
        q_sb = [core.tile([DH, B * NQ], F32, tag=f"q{h}", name=f"q{h}")
                for h in range(H)]
        k_sb = [core.tile([DH, B * NLOC], F32, tag=f"k{h}", name=f"k{h}")
                for h in range(H)]

        # ---- phase A: loads + k head projections ----
        with tc.tile_pool(name="phaseA", bufs=1) as pa:
            for c in range(N_CORES):
                nc.sync.dma_start(
                    out=ctxT[c][:],
                    in_=_view(packed, GATH_BYTES + c * (DH * B * NLOC * 2),
                              BF16, DH, B * NLOC))
            wk_sb = [pa.tile([DH, D], BF16, tag=f"wk{c}", name=f"wk{c}")
                     for c in range(N_CORES)]
            for c in range(N_CORES):
                base = c * GATH_BYTES
                nc.sync.dma_start(
                    out=q_sb[c][:],
                    in_=_view(g_all, base, F32, DH, B * NQ))
                nc.sync.dma_start(
                    out=wk_sb[c][:],
                    in_=_view(g_all, base + QT_BYTES, BF16, DH, D))
                nc.sync.dma_start(
                    out=wv1_sb[c][:],
                    in_=_view(g_all, base + QT_BYTES + WK_BYTES, BF16, DH, NQR))

            for h in range(H):
                kp = ps_m.tile([DH, B * NLOC], F32, tag="m", name="m_ps")
                for c in range(N_CORES):
                    nc.tensor.matmul(kp[:], wk_sb[c][:, h * DH:(h + 1) * DH],
                                     ctxT[c][:], start=(c == 0),
                                     stop=(c == N_CORES - 1))
                nc.scalar.copy(out=k_sb[h][:], in_=kp[:])

        # ---- phase B: h1 + attention partial sums ----
        t_all = dram.tile([128, 4, B, 66], F32)
        with tc.tile_pool(name="phaseB", bufs=1) as pb:
            # SBUF staging partitions = (i_l 4, v 32), v < 8 (= h) is live;
            # compute-engine APs must start at partition 0/32/64/96, so
            # queries sit on 32-row boundaries here and the compaction DMAs
            # below re-pack to (il, h) rows.
            t2_stage = pb.tile([128, 16, B, 66], F32, tag="t2", name="t2")

            def emit_h1(bb):
                h1_t = pb.tile([128, NQR + 2], F32R, tag=f"h1_{bb % 2}",
                               name=f"h1_{bb % 2}")
                stats = small.tile([128, 8, 6], F32, tag="stats", name="stats")
                for nn in range(8):
                    hp = ps_h.tile([128, 512], F32, tag="h_ps", name="h_ps")
                    for c in range(N_CORES):
                        nc.tensor.matmul(
                            hp[:], ctxT[c][:, bb * 128:(bb + 1) * 128],
                            wv1_sb[c][:, nn * 512:(nn + 1) * 512],
                            start=(c == 0), stop=(c == N_CORES - 1))
                    nc.vector.bn_stats(out=stats[:, nn, :], in_=hp[:])
                    nc.scalar.copy(out=h1_t[:, nn * 512:(nn + 1) * 512], in_=hp[:])
                mv = small.tile([128, 2], F32, tag="mv", name="mv")
                nc.vector.bn_aggr(out=mv[:], in_=stats[:])
                # cols 4096/4097: 1/rstd = sqrt(var+eps), mu
                nc.scalar.activation(out=h1_t[:, NQR:NQR + 1], in_=mv[:, 1:2],
                                     func=AF.Sqrt, bias=eps_t[:])
                nc.vector.tensor_copy(out=h1_t[:, NQR + 1:NQR + 2], in_=mv[:, 0:1])
                lnr = small.tile([128, 1], F32, tag="lnr", name="lnr")
                nc.scalar.activation(out=lnr[:], in_=mv[:, 1:2], func=AF.Ln,
                                     bias=eps_t[:])
                nc.vector.tensor_scalar_mul(lnr[:], lnr[:], -0.5)
                return h1_t, lnr

            def emit_scores(bb, lnr):
                # e2 col = i*32 + h (h < 8; cols h >= 8 are never-read junk)
                e2 = pb.tile([128, NQ * 32], F32R, tag="e2", name="e2")
                e2v = e2[:].rearrange("p (i v) -> p i v", v=32)
                for h in range(H):
                    sp = ps_m.tile([128, NQ], F32, tag="m", name="m_ps")
                    nc.tensor.matmul(sp[:], k_sb[h][:, bb * 128:(bb + 1) * 128],
                                     q_sb[h][:, bb * NQ:(bb + 1) * NQ],
                                     start=True, stop=True)
                    nc.scalar.activation(out=e2v[:, :, h], in_=sp[:], func=AF.Exp,
                                         scale=QK_SCALE, bias=lnr[:])
                return e2

            def emit_t5(bb, h1_t, e2):
                # t_raw chunks: 4 queries per matmul, psum partition=(i_l, v32)
                for ic in range(16):
                    tp = ps_t.tile([128, 256], F32, tag="t_ps", name="t_ps")
                    lhs = e2[:, ic * 128:(ic + 1) * 128]
                    nc.tensor.matmul(tp[:], lhs,
                                     h1_t[:, ic * 256:(ic + 1) * 256],
                                     start=True, stop=True)
                    scp = ps_m.tile([128, 2], F32, tag="m", name="m_ps")
                    nc.tensor.matmul(scp[:], lhs, h1_t[:, NQR:NQR + 2],
                                     start=True, stop=True)
                    nc.vector.tensor_copy(out=t2_stage[:, ic, bb, 64:66],
                                          in_=scp[:])
                    for il in range(4):
                        src_ap = tp[il * 32:il * 32 + 8,
                                    il * 64:(il + 1) * 64]
                        dst_ap = t2_stage[il * 32:il * 32 + 8, ic, bb, 0:64]
                        if (ic % 2) == 1:
                            nc.scalar.copy(out=dst_ap, in_=src_ap)
                        else:
                            nc.vector.tensor_copy(out=dst_ap, in_=src_ap)

            # software pipeline: PE fills the stats->exp gap of batch bb with
            # h1 matmuls of batch bb+1
            h1_cur, lnr_cur = emit_h1(0)
            e2_cur = emit_scores(0, lnr_cur)
            for bb in range(B):
                if bb + 1 < B:
                    h1_nxt, lnr_nxt = emit_h1(bb + 1)
                emit_t5(bb, h1_cur, e2_cur)
                if bb + 1 < B:
                    e2_cur = emit_scores(bb + 1, lnr_nxt)
                    h1_cur = h1_nxt

            # compact (i_l, v32) staging into (il, h) rows; plain
            # slices only (partition-split rearranges on DMA operands are
            # silently wrong on this stack)
            for ic in range(16):
                for il in range(4):
                    i = ic * 4 + il
                    row = (i % 16) * 8
                    ig = i // 16
                    nc.sync.dma_start(
                        out=t_all[row:row + 8, ig, :, :],
                        in_=t2_stage[il * 32:il * 32 + 8, ic, :, :])

        # ---- ReduceScatter over the query axis; core c owns rows 16c..16c+15,
        # i.e. queries i with i%16 in {2c, 2c+1} ----
        t_red = dram.tile([16, 4, B, 66], F32)
        nc.gpsimd.collective_compute(
            "ReduceScatter", mybir.AluOpType.add,
            replica_groups=[list(range(N_CORES))],
            ins=[t_all[:].opt()], outs=[t_red[:].opt()])
        nc.sync.dma_start(out=tout[:], in_=t_red[:])


_CACHE = {}


def _get_nc():
    if "nc" not in _CACHE:
        nc = WaitSplitBass("TRN2", target_bir_lowering=False, debug=False,
                           num_devices=N_CORES)
        _CACHE["nc"] = _emit(nc)
    return _CACHE["nc"]


def make_in_maps(x, context, Wq, Wk, Wv1, ln_g, ln_b, Wc, Wout):
    x2 = np.ascontiguousarray(x, dtype=np.float32).reshape(B * NQ, D)
    Wq = np.asarray(Wq, dtype=np.float32)
    qT = np.ascontiguousarray((x2 @ Wq).T)  # [D, B*NQ] f32
    wk_bf = np.asarray(Wk, dtype=np.float32).astype(NP_BF16)
    wv1_bf = np.asarray(Wv1, dtype=np.float32).astype(NP_BF16)
    context = np.asarray(context, dtype=np.float32)
    maps = []
    for c in range(N_CORES):
        ctx_loc = context[:, c * NLOC:(c + 1) * NLOC, :]  # [B, 128, D]
        ctxT = np.ascontiguousarray(
            ctx_loc.transpose(2, 0, 1).reshape(D, B * NLOC).astype(NP_BF16))
        buf = np.empty(PACK_BYTES, dtype=np.uint8)
        o = 0
        for arr in (qT[c * DH:(c + 1) * DH], wk_bf[c * DH:(c + 1) * DH],
                    wv1_bf[c * DH:(c + 1) * DH], ctxT):
            bb = np.ascontiguousarray(arr).view(np.uint8).reshape(-1)
            buf[o:o + bb.size] = bb
            o += bb.size
        assert o == PACK_BYTES
        maps.append({"packed": buf})
    return maps


def assemble(results, ln_g, ln_b, Wc, Wout):
    # stitch the 8 ReduceScattered slices: core c rows = (m 2, h 8) for
    # queries i = 16*ig + 2c + m
    T = np.empty((8, 2, H, 4, B, 66), dtype=np.float32)
    for c in range(N_CORES):
        T[c] = results[c]["tout"].reshape(2, H, 4, B, 66)
    t_raw = T[..., 0:64]                       # sum_j e2 * h1_raw
    se = T[..., 64:65]                         # sum_j exp(s)
    sm = T[..., 65:66]                         # sum_j e2 * mu
    tn = (t_raw - sm) / se                     # sum_j attn * h1_norm
    # [c, m, h, ig, b, r] -> [b, h, (ig, c, m) = i, r]
    tn = np.ascontiguousarray(tn.transpose(4, 2, 3, 0, 1, 5)).reshape(
        B, H, NQ, R)
    g2 = np.asarray(ln_g, dtype=np.float32).reshape(NQ, R)
    b2 = np.asarray(ln_b, dtype=np.float32).reshape(NQ, R)
    mid = tn * g2[None, None] + b2[None, None]
    Wc4 = np.asarray(Wc, dtype=np.float32).reshape(NQ, R, H, DH)
    o = np.einsum("bhir,irhc->bihc", mid, Wc4, optimize=True).reshape(B, NQ, D)
    y = o @ np.asarray(Wout, dtype=np.float32)
    return y.astype(np.float32)


def kernel(x, context, Wq, Wk, Wv1, ln_g, ln_b, Wc, Wout):
    nc = _get_nc()
    maps = make_in_maps(x, context, Wq, Wk, Wv1, ln_g, ln_b, Wc, Wout)
    res = run_bass_kernel_spmd(nc, maps, list(range(N_CORES)))
    # guard against a transient all-zero result (sumexp must be positive);
    # re-dispatch once rather than emit NaN/garbage
    if not all(np.all(r["tout"][:, :, :, 64] > 0) for r in res.results):
        res = run_bass_kernel_spmd(nc, maps, list(range(N_CORES)))
    return assemble(res.results, ln_g, ln_b, Wc, Wout)


# revision 9
# speedup vs baseline: 10.5661x; 1.1593x over previous
"""Trainium2 Bass kernel for nn_CrossAttention (q-aware per-query V cross attention).

Reference computation (b=4, nq=64, n=1024, d=768, h=8, dh=96, R=64):
    q   = x @ Wq
    k   = context @ Wk
    h1  = LayerNorm(context @ Wv1)            # over the 4096 (= nq*R) axis
    vmid= h1.reshape(b, n, nq, R)
    v   = einsum('bnqr,qrd->bqnd', vmid, Wc)
    attn= softmax(q·k / sqrt(dh))             # per head
    out = einsum('bhij,bhijd->bhid', attn, v) @ Wout

Key algebraic restructuring: contract attn with vmid FIRST
(t[b,i,h,r] = sum_j attn[b,h,i,j] * vmid[b,j,i,r]), then apply the grouped
conv Wc and Wout on the tiny rank-space result. This avoids materializing
the 805MB v tensor and collapses ~52 GFLOP to ~6 GFLOP.

This version is engineered for the axon-tunnel regime where host->device
transfer (~90MB/s marginal, ~7ms fixed per array) dominates wall clock.
Per-call wire traffic is cut from ~189MB to ~11MB:
  * every replicated tensor is sharded 1/8 per core (96 = D/8 rows each of
    q^T, Wk, Wv1) and AllGathered on-device over NeuronLink. The 96-row
    shard boundaries double as the matmul contraction chunking, so the
    gathered regions are consumed in place with zero repacking;
  * one packed u8 wire tensor per core ([qT | Wk | Wv1 | ctxT]) to pay the
    per-array fixed cost once; one staging DMA + one AllGather covers the
    first three regions, ctxT stays core-local;
  * bf16 on the wire for ctx/Wk/Wv1 (q^T stays f32 as the precision
    anchor of the score path). fp8 was measured at 2.7e-2 absmax-rel for
    either ctx or Wv1 (host-only ablation matches) vs 2.5e-3 for all-bf16,
    so bf16 is the wire floor;
  * the tiny endpoints run on host: q = x@Wq (151 MFLOP) before dispatch,
    and the rank-space tail (gamma/beta fold, grouped conv Wc, Wout:
    ~330 MFLOP) after summing the 8 partial t tensors (135KB each).

Device work per core: k head projections, h1 = ctx_loc @ Wv1 (25.8 GFLOP
fleet-wide), LN stats folded into the exp bias (e2 = exp(s)*rstd via
ln(rstd) bias; 1/rstd and mu appended as h1 columns so one matmul yields
both normalizers), and t = e2^T @ h1 partial sums over the local 128
context rows. Host reduces the 8 partials (the old on-device
ReduceScatter) and finishes the tail in numpy.
"""

import json
import os

import numpy as np
import ml_dtypes

import jax

# Fresh shard_map closures inside run_bass_kernel_spmd defeat jax's
# in-memory executable cache, so every call re-runs the BIR->NEFF pipeline
# (~0.35s). The persistent cache is keyed on the (stable) HLO hash and
# brings repeat calls down to a disk load.
jax.config.update("jax_compilation_cache_dir",
                  os.path.expanduser("~/.cache/jax_bass_cache"))
jax.config.update("jax_persistent_cache_min_entry_size_bytes", -1)
jax.config.update("jax_persistent_cache_min_compile_time_secs", 0.0)

import concourse.bass as bass
import concourse.mybir as mybir
import concourse.tile as tile
from concourse.bass_utils import run_bass_kernel_spmd

F32 = mybir.dt.float32
F32R = mybir.dt.float32r
BF16 = mybir.dt.bfloat16
FP8 = mybir.dt.float8e4
AF = mybir.ActivationFunctionType
NP_BF16 = ml_dtypes.bfloat16
NP_FP8 = ml_dtypes.float8_e4m3

B = 4
NQ = 64
N = 1024
D = 768
H = 8
DH = 96
R = 64
NQR = NQ * R  # 4096
LN_EPS = 1e-5
N_CORES = 8
NLOC = N // N_CORES  # 128 context rows per batch per core
QK_SCALE = float(DH) ** -0.5

# packed wire layout per core (bytes)
QT_BYTES = DH * B * NQ * 4          # 98304  f32 [96, 256]
WK_BYTES = DH * D * 2               # 147456 bf16 [96, 768]
WV1_BYTES = DH * NQR * 2            # 786432 bf16 [96, 4096]
GATH_BYTES = QT_BYTES + WK_BYTES + WV1_BYTES  # 1032192, AllGathered
CTX_BYTES = D * B * NLOC * 2        # 786432 bf16 [8, 96, 512], core-local
PACK_BYTES = GATH_BYTES + CTX_BYTES


class WaitSplitBass(bass.Bass):
    """This walrus build rejects instructions carrying more than one sync
    wait; split extras into preceding same-engine NoOps at JSON time."""

    MAX_WAITS = 1
    _json_memo = None

    def to_json_bytes(self) -> bytes:
        # the module is immutable once _emit() returns; serializing it costs
        # ~40ms per call inside the jit lowering, so memoize
        if self._json_memo is not None:
            return self._json_memo
        self._json_memo = self._to_json_bytes_impl()
        return self._json_memo

    def _to_json_bytes_impl(self) -> bytes:
        raw = super().to_json_bytes()
        m = json.loads(raw)
        changed = False
        for f in m.get("functions", []):
            for blk in f.get("blocks", []):
                out = []
                for inst in blk.get("instructions", []):
                    si = inst.get("sync_info")
                    waits = si.get("on_wait") if si else None
                    if waits and len(waits) > self.MAX_WAITS:
                        extra = waits[self.MAX_WAITS:]
                        si["on_wait"] = waits[: self.MAX_WAITS]
                        for k, w in enumerate(extra):
                            out.append({
                                "engine": inst["engine"],
                                "ins": [],
                                "name": f"{inst['name']}_ws{k}",
                                "opcode": "NoOp",
                                "outs": [],
                                "sync_info": {"on_update": [], "on_wait": [w]},
                            })
                        changed = True
                    out.append(inst)
                blk["instructions"] = out
        return json.dumps(m).encode() if changed else raw


def _emit(nc):
    packed = nc.declare_dram_parameter("packed", [PACK_BYTES], mybir.dt.uint8,
                                       isOutput=False)
    # ReduceScattered t sums: rows (m 2, h 8) -> query i = 16*ig + 2*core + m
    tout = nc.declare_dram_parameter("tout", [16, 4, B, 66], F32,
                                     isOutput=True)
    with tile.TileContext(nc) as tc:
        _body(nc, tc, packed, tout)
    return nc


def _view(t, byte_off, dtype, rows, cols):
    """2-D [rows, cols] view of a byte range of the flat u8 tensor t."""
    esz = np.dtype(mybir.dt.np(dtype)).itemsize
    ap = t[byte_off:byte_off + rows * cols * esz].bitcast(dtype)
    return ap.rearrange("(a b) -> a b", a=rows)


def _body(nc, tc, packed, tout):
    from contextlib import ExitStack

    with ExitStack() as st:
        const = st.enter_context(tc.tile_pool(name="const", bufs=1))
        core = st.enter_context(tc.tile_pool(name="core", bufs=1))
        small = st.enter_context(tc.tile_pool(name="small", bufs=4))
        ps_h = st.enter_context(tc.tile_pool(name="ps_h", bufs=2, space="PSUM"))
        ps_m = st.enter_context(tc.tile_pool(name="ps_m", bufs=2, space="PSUM"))
        ps_t = st.enter_context(tc.tile_pool(name="ps_t", bufs=2, space="PSUM"))
        dram = st.enter_context(tc.tile_pool(name="dram", bufs=1, space="DRAM"))

        eps_t = const.tile([128, 1], F32)
        nc.vector.memset(eps_t[:], LN_EPS)

        # ---- one staged copy + one AllGather of the [qT|Wk|Wv1] regions ----
        # (collectives cannot read IO tensors, hence the staging DMA)
        s_all = dram.tile([GATH_BYTES], mybir.dt.uint8)
        g_all = dram.tile([N_CORES * GATH_BYTES], mybir.dt.uint8,
                          addr_space="Shared")
        nc.sync.dma_start(out=s_all[:], in_=packed[0:GATH_BYTES])
        nc.gpsimd.collective_compute(
            "AllGather", mybir.AluOpType.bypass,
            replica_groups=[list(range(N_CORES))],
            ins=[s_all[:].opt()], outs=[g_all[:].opt()])

        # core-resident tensors
        wv1_sb = [core.tile([DH, NQR], BF16, tag=f"wv1{c}", name=f"wv1{c}")
                  for c in range(N_CORES)]
        ctxT = [core.tile([DH, B * NLOC], BF16, tag=f"cT{c}", name=f"cT{c}")
                for c in range(N_CORES)]
        q_sb = [core.tile([DH, B * NQ], F32, tag=f"q{h}", name=f"q{h}")
                for h in range(H)]
        k_sb = [core.tile([DH, B * NLOC], F32, tag=f"k{h}", name=f"k{h}")
                for h in range(H)]

        # ---- phase A: loads + k head projections ----
        with tc.tile_pool(name="phaseA", bufs=1) as pa:
            for c in range(N_CORES):
                nc.sync.dma_start(
                    out=ctxT[c][:],
                    in_=_view(packed, GATH_BYTES + c * (DH * B * NLOC * 2),
                              BF16, DH, B * NLOC))
            wk_sb = [pa.tile([DH, D], BF16, tag=f"wk{c}", name=f"wk{c}")
                     for c in range(N_CORES)]
            for c in range(N_CORES):
                base = c * GATH_BYTES
                nc.sync.dma_start(
                    out=q_sb[c][:],
                    in_=_view(g_all, base, F32, DH, B * NQ))
                nc.sync.dma_start(
                    out=wk_sb[c][:],
                    in_=_view(g_all, base + QT_BYTES, BF16, DH, D))
                nc.sync.dma_start(
                    out=wv1_sb[c][:],
                    in_=_view(g_all, base + QT_BYTES + WK_BYTES, BF16, DH, NQR))

            for h in range(H):
                kp = ps_m.tile([DH, B * NLOC], F32, tag="m", name="m_ps")
                for c in range(N_CORES):
                    nc.tensor.matmul(kp[:], wk_sb[c][:, h * DH:(h + 1) * DH],
                                     ctxT[c][:], start=(c == 0),
                                     stop=(c == N_CORES - 1))
                nc.scalar.copy(out=k_sb[h][:], in_=kp[:])

        # ---- phase B: h1 + attention partial sums ----
        t_all = dram.tile([128, 4, B, 66], F32)
        with tc.tile_pool(name="phaseB", bufs=1) as pb:
            # SBUF staging partitions = (i_l 4, v 32), v < 8 (= h) is live;
            # compute-engine APs must start at partition 0/32/64/96, so
            # queries sit on 32-row boundaries here and the compaction DMAs
            # below re-pack to (il, h) rows.
            t2_stage = pb.tile([128, 16, B, 66], F32, tag="t2", name="t2")

            def emit_h1(bb):
                h1_t = pb.tile([128, NQR + 2], F32R, tag=f"h1_{bb % 2}",
                               name=f"h1_{bb % 2}")
                stats = small.tile([128, 8, 6], F32, tag="stats", name="stats")
                for nn in range(8):
                    hp = ps_h.tile([128, 512], F32, tag="h_ps", name="h_ps")
                    for c in range(N_CORES):
                        nc.tensor.matmul(
                            hp[:], ctxT[c][:, bb * 128:(bb + 1) * 128],
                            wv1_sb[c][:, nn * 512:(nn + 1) * 512],
                            start=(c == 0), stop=(c == N_CORES - 1))
                    nc.vector.bn_stats(out=stats[:, nn, :], in_=hp[:])
                    nc.scalar.copy(out=h1_t[:, nn * 512:(nn + 1) * 512], in_=hp[:])
                mv = small.tile([128, 2], F32, tag="mv", name="mv")
                nc.vector.bn_aggr(out=mv[:], in_=stats[:])
                # cols 4096/4097: 1/rstd = sqrt(var+eps), mu
                nc.scalar.activation(out=h1_t[:, NQR:NQR + 1], in_=mv[:, 1:2],
                                     func=AF.Sqrt, bias=eps_t[:])
                nc.vector.tensor_copy(out=h1_t[:, NQR + 1:NQR + 2], in_=mv[:, 0:1])
                lnr = small.tile([128, 1], F32, tag="lnr", name="lnr")
                nc.scalar.activation(out=lnr[:], in_=mv[:, 1:2], func=AF.Ln,
                                     bias=eps_t[:])
                nc.vector.tensor_scalar_mul(lnr[:], lnr[:], -0.5)
                return h1_t, lnr

            def emit_scores(bb, lnr):
                # e2 col = i*32 + h (h < 8; cols h >= 8 are never-read junk)
                e2 = pb.tile([128, NQ * 32], F32R, tag="e2", name="e2")
                e2v = e2[:].rearrange("p (i v) -> p i v", v=32)
                for h in range(H):
                    sp = ps_m.tile([128, NQ], F32, tag="m", name="m_ps")
                    nc.tensor.matmul(sp[:], k_sb[h][:, bb * 128:(bb + 1) * 128],
                                     q_sb[h][:, bb * NQ:(bb + 1) * NQ],
                                     start=True, stop=True)
                    nc.scalar.activation(out=e2v[:, :, h], in_=sp[:], func=AF.Exp,
                                         scale=QK_SCALE, bias=lnr[:])
                return e2

            def emit_t5(bb, h1_t, e2):
                # t_raw chunks: 4 queries per matmul, psum partition=(i_l, v32)
                for ic in range(16):
                    tp = ps_t.tile([128, 256], F32, tag="t_ps", name="t_ps")
                    lhs = e2[:, ic * 128:(ic + 1) * 128]
                    nc.tensor.matmul(tp[:], lhs,
                                     h1_t[:, ic * 256:(ic + 1) * 256],
                                     start=True, stop=True)
                    scp = ps_m.tile([128, 2], F32, tag="m", name="m_ps")
                    nc.tensor.matmul(scp[:], lhs, h1_t[:, NQR:NQR + 2],
                                     start=True, stop=True)
                    nc.vector.tensor_copy(out=t2_stage[:, ic, bb, 64:66],
                                          in_=scp[:])
                    for il in range(4):
                        src_ap = tp[il * 32:il * 32 + 8,
                                    il * 64:(il + 1) * 64]
                        dst_ap = t2_stage[il * 32:il * 32 + 8, ic, bb, 0:64]
                        if (ic % 2) == 1:
                            nc.scalar.copy(out=dst_ap, in_=src_ap)
                        else:
                            nc.vector.tensor_copy(out=dst_ap, in_=src_ap)

            # software pipeline: PE fills the stats->exp gap of batch bb with
            # h1 matmuls of batch bb+1
            h1_cur, lnr_cur = emit_h1(0)
            e2_cur = emit_scores(0, lnr_cur)
            for bb in range(B):
                if bb + 1 < B:
                    h1_nxt, lnr_nxt = emit_h1(bb + 1)
                emit_t5(bb, h1_cur, e2_cur)
                if bb + 1 < B:
                    e2_cur = emit_scores(bb + 1, lnr_nxt)
                    h1_cur = h1_nxt

            # compact (i_l, v32) staging into (il, h) rows; plain
            # slices only (partition-split rearranges on DMA operands are
            # silently wrong on this stack)
            for ic in range(16):
                for il in range(4):
                    i = ic * 4 + il
                    row = (i % 16) * 8
                    ig = i // 16
                    nc.sync.dma_start(
                        out=t_all[row:row + 8, ig, :, :],
                        in_=t2_stage[il * 32:il * 32 + 8, ic, :, :])

        # ---- ReduceScatter over the query axis; core c owns rows 16c..16c+15,
        # i.e. queries i with i%16 in {2c, 2c+1} ----
        t_red = dram.tile([16, 4, B, 66], F32)
        nc.gpsimd.collective_compute(
            "ReduceScatter", mybir.AluOpType.add,
            replica_groups=[list(range(N_CORES))],
            ins=[t_all[:].opt()], outs=[t_red[:].opt()])
        nc.sync.dma_start(out=tout[:], in_=t_red[:])


_CACHE = {}


def _get_nc():
    if "nc" not in _CACHE:
        nc = WaitSplitBass("TRN2", target_bir_lowering=False, debug=False,
                           num_devices=N_CORES)
        _CACHE["nc"] = _emit(nc)
    return _CACHE["nc"]


def make_in_maps(x, context, Wq, Wk, Wv1, ln_g, ln_b, Wc, Wout):
    x2 = np.ascontiguousarray(x, dtype=np.float32).reshape(B * NQ, D)
    Wq = np.asarray(Wq, dtype=np.float32)
    qT = np.ascontiguousarray((x2 @ Wq).T)  # [D, B*NQ] f32
    wk_bf = np.asarray(Wk, dtype=np.float32).astype(NP_BF16)
    wv1_bf = np.asarray(Wv1, dtype=np.float32).astype(NP_BF16)
    context = np.asarray(context, dtype=np.float32)
    maps = []
    for c in range(N_CORES):
        ctx_loc = context[:, c * NLOC:(c + 1) * NLOC, :]  # [B, 128, D]
        ctxT = np.ascontiguousarray(
            ctx_loc.transpose(2, 0, 1).reshape(D, B * NLOC).astype(NP_BF16))
        buf = np.empty(PACK_BYTES, dtype=np.uint8)
        o = 0
        for arr in (qT[c * DH:(c + 1) * DH], wk_bf[c * DH:(c + 1) * DH],
                    wv1_bf[c * DH:(c + 1) * DH], ctxT):
            bb = np.ascontiguousarray(arr).view(np.uint8).reshape(-1)
            buf[o:o + bb.size] = bb
            o += bb.size
        assert o == PACK_BYTES
        maps.append({"packed": buf})
    return maps


def assemble(results, ln_g, ln_b, Wc, Wout):
    # stitch the 8 ReduceScattered slices: core c rows = (m 2, h 8) for
    # queries i = 16*ig + 2c + m
    T = np.empty((8, 2, H, 4, B, 66), dtype=np.float32)
    for c in range(N_CORES):
        T[c] = results[c]["tout"].reshape(2, H, 4, B, 66)
    t_raw = T[..., 0:64]                       # sum_j e2 * h1_raw
    se = T[..., 64:65]                         # sum_j exp(s)
    sm = T[..., 65:66]                         # sum_j e2 * mu
    tn = (t_raw - sm) / se                     # sum_j attn * h1_norm
    # [c, m, h, ig, b, r] -> [b, h, (ig, c, m) = i, r]
    tn = np.ascontiguousarray(tn.transpose(4, 2, 3, 0, 1, 5)).reshape(
        B, H, NQ, R)
    g2 = np.asarray(ln_g, dtype=np.float32).reshape(NQ, R)
    b2 = np.asarray(ln_b, dtype=np.float32).reshape(NQ, R)
    mid = tn * g2[None, None] + b2[None, None]
    Wc4 = np.asarray(Wc, dtype=np.float32).reshape(NQ, R, H, DH)
    o = np.einsum("bhir,irhc->bihc", mid, Wc4, optimize=True).reshape(B, NQ, D)
    y = o @ np.asarray(Wout, dtype=np.float32)
    return y.astype(np.float32)


def kernel(x, context, Wq, Wk, Wv1, ln_g, ln_b, Wc, Wout):
    nc = _get_nc()
    maps = make_in_maps(x, context, Wq, Wk, Wv1, ln_g, ln_b, Wc, Wout)
    res = run_bass_kernel_spmd(nc, maps, list(range(N_CORES)))
    # guard against a transient all-zero result (sumexp must be positive);
    # re-dispatch once rather than emit NaN/garbage
    if not all(np.all(r["tout"][:, :, :, 64] > 0) for r in res.results):
        res = run_bass_kernel_spmd(nc, maps, list(range(N_CORES)))
    return assemble(res.results, ln_g, ln_b, Wc, Wout)
